# revision 2
# baseline (speedup 1.0000x reference)
"""ConvLSTM2D (Keras gates, hard_sigmoid) + inference BatchNorm on 8 trn2 cores.

Sharding: batch(2) x H-slabs(4). Each core owns 16 rows. Buffer holds
own +/- 8 halo rows (32 rows). Steps 1-8 shrink the halo 30->16 rows; one
AllToAll among all 8 cores refreshes the 8-row halos with h_8 from the
ring neighbors; steps 9-16 shrink again. This computes ~23 rows/step
instead of the 31 a full 16-step shrinking halo needs.

Layout: channels-on-partitions, f16. zin [128, 2114]: x_t on partitions
0-63, h_{t-1} on 64-127; rows are 66 cols (64 + guard col each side) plus
one pad col at each buffer end so corner taps stay in bounds. Conv =
9 taps x 2 gate-halves of accumulating f16 matmuls (128-contraction packs
x and h channels together). Gate math: hard_sigmoid scale 0.2 folded into
the i/f/o weight columns; Act does Relu(ps+B) / Tanh; DVE f16 ops do
clip/products; min(o, mask) applies the top clip and the validity mask in
one op (mask is {0,1}), so h lands already masked in the next zin buffer.
Output: h in f16 per step; BN (+f32 cast) applied on the host.
"""
import math
import numpy as np

import concourse.bass as bass
import concourse.mybir as mybir
import concourse.tile as tile
from concourse.bass_utils import run_bass_kernel_spmd

F16 = np.float16
F32 = np.float32

T, F, C, W = 16, 64, 64, 64
HALO = 8
NROW = 16 + 2 * HALO          # 32 buffer rows
WP = W + 2                    # 66
NCOL = NROW * WP + 2          # 2114 (pad col at each end)
COL0 = 1                      # first row starts at col 1
CHUNK = 67584                 # 2 * 64ch * 8rows * 66cols: [h|c] slot elems
TAPS = [(dy, dx) for dy in (-1, 0, 1) for dx in (-1, 0, 1)]

TRACE_SIM = False
_PROG = None
_LAST_TC = None

_MAX_WAITS = 1


def _split_multi_waits(nc):
    for fn in nc.m.functions:
        for bb in fn.blocks:
            lst = bb.instructions
            out, changed = [], False
            for ins in lst:
                si = ins.sync_info
                if si is not None and len(si.on_wait) > _MAX_WAITS:
                    waits = list(si.on_wait)
                    extra, keep = waits[:-_MAX_WAITS], waits[-_MAX_WAITS:]
                    for j, w in enumerate(extra):
                        nop = mybir.InstNoOp(
                            name=f"{ins.name}.sw{j}", ins=[], outs=[],
                            text_hint="split_wait", bass_nofuse=True)
                        nop.engine = ins.engine
                        nop.sync_info = mybir.SyncInfo(on_wait=[w], on_update=[])
                        out.append(nop)
                    ins.sync_info = mybir.SyncInfo(
                        on_wait=keep, on_update=list(si.on_update))
                    changed = True
                out.append(ins)
            if changed:
                try:
                    bb.instructions = out
                except Exception:
                    lst.clear()
                    lst.extend(out)


def _chunks(c0, c1, maxn=512):
    L = c1 - c0
    n = max(1, math.ceil(L / maxn))
    base, rem = divmod(L, n)
    sizes = [base + (1 if i < rem else 0) for i in range(n)]
    out, p = [], c0
    for s in sizes:
        out.append((p, s))
        p += s
    return out


def _build():
    nc = bass.Bass(target_bir_lowering=False)
    f32, f16 = mybir.dt.float32, mybir.dt.float16
    i32 = mybir.dt.int32

    xT_d = nc.dram_tensor("xT", [T, C, NCOL], f16, kind="ExternalInput")
    w_d = nc.dram_tensor("w", [128, 18 * 128], f16, kind="ExternalInput")
    mask_d = nc.dram_tensor("mask", [128, NCOL], f16, kind="ExternalInput")
    bif_d = nc.dram_tensor("bif", [128, 1], f32, kind="ExternalInput")
    bg_d = nc.dram_tensor("bg", [F, 1], f32, kind="ExternalInput")
    bo_d = nc.dram_tensor("bo", [F, 1], f32, kind="ExternalInput")
    slots_d = nc.dram_tensor("slots", [1, 2], i32, kind="ExternalInput")
    out_d = nc.dram_tensor("out", [T, F, 16 * WP], f16, kind="ExternalOutput")
    ccin_d = nc.dram_tensor("cc_in", [8 * CHUNK], f16, kind="Internal")
    ccout_d = nc.dram_tensor("cc_out", [8 * CHUNK], f16, kind="Internal")

    Relu = mybir.ActivationFunctionType.Relu
    Tanh = mybir.ActivationFunctionType.Tanh
    AL = mybir.AluOpType

    with tile.TileContext(nc, trace_sim=TRACE_SIM) as tc:
        with (
            tc.tile_pool(name="const", bufs=1) as cpool,
            tc.tile_pool(name="state", bufs=1) as spool,
            tc.tile_pool(name="work", bufs=3) as wpool,
            tc.psum_pool(name="ps", bufs=3) as pspool,
        ):
            w_sb = cpool.tile([128, 18 * 128], f16)
            mask_sb = cpool.tile([128, NCOL], f16)
            bif_sb = cpool.tile([128, 1], f32)
            bg_sb = cpool.tile([F, 1], f32)
            bo_sb = cpool.tile([F, 1], f32)
            slots_sb = cpool.tile([1, 2], i32)
            nc.sync.dma_start(w_sb[:], w_d[:])
            nc.sync.dma_start(mask_sb[:], mask_d[:])
            nc.sync.dma_start(bif_sb[:], bif_d[:])
            nc.sync.dma_start(bg_sb[:], bg_d[:])
            nc.sync.dma_start(bo_sb[:], bo_d[:])
            nc.sync.dma_start(slots_sb[:], slots_d[:])

            zin = [spool.tile([128, NCOL], f16, name=f"zin{i}", tag=f"zin{i}")
                   for i in range(2)]
            gc = spool.tile([128, NCOL], f16, tag="gc")  # g on 0:64, c on 64:128
            nc.vector.memset(zin[0][:, :], 0.0)
            nc.vector.memset(zin[1][:, :], 0.0)
            nc.vector.memset(gc[:, :], 0.0)

            up_off = nc.gpsimd.alloc_register()
            dn_off = nc.gpsimd.alloc_register()
            nc.gpsimd.reg_load(up_off, slots_sb[0:1, 0:1])
            nc.gpsimd.reg_load(dn_off, slots_sb[0:1, 1:2])

            for t in range(1, T + 1):
                j = ((t - 1) % 8) + 1          # step within phase: 1..8
                cur = zin[(t - 1) % 2]
                nxt = zin[t % 2]
                # x_t rows [j-1, 33-j)
                xc0 = COL0 + (j - 1) * WP
                xc1 = COL0 + (NROW + 1 - j) * WP
                nc.sync.dma_start(cur[0:64, xc0:xc1], xT_d[t - 1, :, xc0:xc1])

                p_lo = COL0 + j * WP
                p_hi = COL0 + (NROW - j) * WP
                for p0, n in _chunks(p_lo, p_hi):
                    ps_if = pspool.tile([128, n], f32, tag="psif")
                    ps_go = pspool.tile([128, n], f32, tag="psgo")
                    for k, (dy, dx) in enumerate(TAPS):
                        off = p0 + dy * WP + dx
                        nc.tensor.matmul(
                            ps_if[:], w_sb[:, k * 128:(k + 1) * 128],
                            cur[:, off:off + n], start=(k == 0), stop=(k == 8))
                    for k, (dy, dx) in enumerate(TAPS):
                        off = p0 + dy * WP + dx
                        nc.tensor.matmul(
                            ps_go[:], w_sb[:, 1152 + k * 128:1152 + (k + 1) * 128],
                            cur[:, off:off + n], start=(k == 0), stop=(k == 8))

                    sif = wpool.tile([128, n], f16, tag="sif")
                    o1 = wpool.tile([F, n], f16, tag="o1")
                    o2 = wpool.tile([F, n], f16, tag="o2")
                    tt = wpool.tile([F, n], f16, tag="tt")
                    t2 = wpool.tile([F, n], f16, tag="t2")
                    tc_t = wpool.tile([F, n], f16, tag="tc")

                    # i,f: Relu(ps + B) then min 1  (0.2 folded into W)
                    nc.scalar.activation(sif[:], ps_if[:], Relu,
                                         bias=bif_sb[:, 0:1], scale=1.0)
                    nc.vector.tensor_scalar_min(sif[:], sif[:], 1.0)
                    # g: tanh(ps + bg) -> gc[0:64]
                    nc.scalar.activation(gc[0:64, p0:p0 + n], ps_go[0:64, :],
                                         Tanh, bias=bg_sb[:, 0:1], scale=1.0)
                    # o: Relu(ps + B); clip top + mask in one min
                    nc.scalar.activation(o1[:], ps_go[64:128, :], Relu,
                                         bias=bo_sb[:, 0:1], scale=1.0)
                    nc.vector.tensor_tensor(o2[:], o1[:], mask_sb[0:64, p0:p0 + n],
                                            AL.min)
                    # c update: t1 = i*g, t2 = f*c (base-aligned pairs); c = t1+t2
                    nc.vector.tensor_mul(tt[0:64, :], sif[0:64, :],
                                         gc[0:64, p0:p0 + n])
                    nc.vector.tensor_mul(t2[:], sif[64:128, :],
                                         gc[64:128, p0:p0 + n])
                    nc.vector.tensor_add(gc[64:128, p0:p0 + n],
                                         tt[0:64, :], t2[:])
                    nc.scalar.activation(tc_t[:], gc[64:128, p0:p0 + n], Tanh)
                    # h = o2 * tanh(c) -> next buffer h-half (masked, guards 0)
                    nc.vector.tensor_mul(nxt[64:128, p0:p0 + n], o2[:], tc_t[:])

                # own rows out (h_t in f16; BN on host)
                oc0 = COL0 + HALO * WP
                nc.sync.dma_start(out_d[t - 1], nxt[64:128, oc0:oc0 + 16 * WP])

                if t == 8:
                    # halo refresh: send own top/bottom 8 rows of h_8,
                    # AllToAll over all 8 cores, read ring-neighbor chunks.
                    top0 = COL0 + HALO * WP
                    mid0 = COL0 + 16 * WP
                    bot1 = COL0 + 24 * WP
                    up_off2 = nc.gpsimd.alloc_register()
                    dn_off2 = nc.gpsimd.alloc_register()
                    nc.gpsimd.reg_add(up_off2, up_off, 33792)
                    nc.gpsimd.reg_add(dn_off2, dn_off, 33792)
                    # to up-peer: slot (s-1): my top 8 own rows of h_8 and c_8
                    ap_up = bass.AP(ccin_d, up_off,
                                    [[528, 64], [1, 528]],
                                    dep_tracking_offset=0)
                    nc.gpsimd.dma_start(ap_up, nxt[64:128, top0:mid0])
                    ap_upc = bass.AP(ccin_d, up_off2,
                                     [[528, 64], [1, 528]],
                                     dep_tracking_offset=33792)
                    nc.gpsimd.dma_start(ap_upc, gc[64:128, top0:mid0])
                    # to down-peer: slot (s+1): my bottom 8 own rows
                    ap_dn = bass.AP(ccin_d, dn_off,
                                    [[528, 64], [1, 528]],
                                    dep_tracking_offset=0)
                    nc.gpsimd.dma_start(ap_dn, nxt[64:128, mid0:bot1])
                    ap_dnc = bass.AP(ccin_d, dn_off2,
                                     [[528, 64], [1, 528]],
                                     dep_tracking_offset=33792)
                    nc.gpsimd.dma_start(ap_dnc, gc[64:128, mid0:bot1])
                    nc.gpsimd.collective_compute(
                        "AllToAll", AL.bypass,
                        replica_groups=[[0, 1, 2, 3, 4, 5, 6, 7]],
                        ins=[ccin_d[:]], outs=[ccout_d[:]])
                    # from up-peer (slot s-1): their bottom rows -> my rows 0..8
                    ro_up = bass.AP(ccout_d, up_off,
                                    [[528, 64], [1, 528]],
                                    dep_tracking_offset=0)
                    nc.gpsimd.dma_start(nxt[64:128, COL0:COL0 + 528], ro_up)
                    ro_upc = bass.AP(ccout_d, up_off2,
                                     [[528, 64], [1, 528]],
                                     dep_tracking_offset=33792)
                    nc.gpsimd.dma_start(gc[64:128, COL0:COL0 + 528], ro_upc)
                    ro_dn = bass.AP(ccout_d, dn_off,
                                    [[528, 64], [1, 528]],
                                    dep_tracking_offset=0)
                    nc.gpsimd.dma_start(nxt[64:128, bot1:bot1 + 528], ro_dn)
                    ro_dnc = bass.AP(ccout_d, dn_off2,
                                     [[528, 64], [1, 528]],
                                     dep_tracking_offset=33792)
                    nc.gpsimd.dma_start(gc[64:128, bot1:bot1 + 528], ro_dnc)
                    # zero out-of-image halo rows (and wrap garbage)
                    nc.vector.tensor_mul(nxt[64:128, COL0:COL0 + 528],
                                         nxt[64:128, COL0:COL0 + 528],
                                         mask_sb[64:128, COL0:COL0 + 528])
                    nc.vector.tensor_mul(nxt[64:128, bot1:bot1 + 528],
                                         nxt[64:128, bot1:bot1 + 528],
                                         mask_sb[64:128, bot1:bot1 + 528])

        global _LAST_TC
        _LAST_TC = tc
    _split_multi_waits(nc)
    return nc


def _prep_inputs(x, Wx, Wh, b, gamma, beta, moving_mean, moving_var):
    x = np.asarray(x, F32)
    Wx = np.asarray(Wx, F32)
    Wh = np.asarray(Wh, F32)
    b = np.asarray(b, F32)
    # fold the hard_sigmoid 0.2 into i,f,o weight columns; g unscaled
    sc = np.ones((256,), F32)
    sc[0:128] = 0.2
    sc[192:256] = 0.2
    Wxs = Wx * sc
    Whs = Wh * sc
    wstack = np.zeros((128, 18 * 128), F32)
    for k, (dy, dx) in enumerate(TAPS):
        ky, kx = dy + 1, dx + 1
        wstack[0:64, k * 128:(k + 1) * 128] = Wxs[ky, kx, :, 0:128]
        wstack[64:128, k * 128:(k + 1) * 128] = Whs[ky, kx, :, 0:128]
        wstack[0:64, 1152 + k * 128:1152 + (k + 1) * 128] = Wxs[ky, kx, :, 128:256]
        wstack[64:128, 1152 + k * 128:1152 + (k + 1) * 128] = Whs[ky, kx, :, 128:256]
    wstack = wstack.astype(F16)

    bif = (0.2 * b[0:128] + 0.5).reshape(128, 1).astype(F32)
    bg = b[128:192].reshape(64, 1).astype(F32)
    bo = (0.2 * b[192:256] + 0.5).reshape(64, 1).astype(F32)

    in_maps = []
    for core in range(8):
        bidx, s = core // 4, core % 4
        r0 = 16 * s
        glo, ghi = max(0, r0 - HALO), min(64, r0 + 16 + HALO)
        i0 = glo - (r0 - HALO)
        xpad = np.zeros((T, NROW, WP, C), F32)
        xpad[:, i0:i0 + (ghi - glo), 1:65, :] = x[bidx, :, glo:ghi, :, :]
        xT = np.zeros((T, C, NCOL), F32)
        xT[:, :, COL0:COL0 + NROW * WP] = (
            xpad.transpose(0, 3, 1, 2).reshape(T, C, NROW * WP))
        xT = xT.astype(F16)
        m = np.zeros((NROW, WP), F32)
        for i in range(NROW):
            if 0 <= (r0 - HALO + i) < 64:
                m[i, 1:65] = 1.0
        mask = np.zeros((NCOL,), F32)
        mask[COL0:COL0 + NROW * WP] = m.reshape(-1)
        mask = np.broadcast_to(mask.reshape(1, NCOL), (128, NCOL)).astype(F16).copy()
        slots = np.array([[((core - 1) % 8) * CHUNK,
                           ((core + 1) % 8) * CHUNK]], np.int32)
        in_maps.append({
            "xT": xT, "w": wstack, "mask": mask, "bif": bif,
            "bg": bg, "bo": bo, "slots": slots,
        })
    return in_maps


def kernel(x, Wx, Wh, b, gamma, beta, moving_mean, moving_var):
    global _PROG
    if _PROG is None:
        _PROG = _build()
    in_maps = _prep_inputs(x, Wx, Wh, b, gamma, beta, moving_mean, moving_var)
    res = run_bass_kernel_spmd(_PROG, in_maps, core_ids=list(range(8)))
    inv = (np.asarray(gamma, F32) /
           np.sqrt(np.asarray(moving_var, F32) + 1e-3))
    bnb = np.asarray(beta, F32) - np.asarray(moving_mean, F32) * inv
    out = np.empty((2, T, 64, W, F), F32)
    for core in range(8):
        bidx, s = core // 4, core % 4
        oc = res.results[core]["out"].astype(F32).reshape(T, F, 16, WP)[:, :, :, 1:65]
        out[bidx, :, 16 * s:16 * s + 16] = oc.transpose(0, 2, 3, 1)
    out = out * inv + bnb
    return out
